# revision 50
# baseline (speedup 1.0000x reference)
"""Distributed causal multi-head attention for 8 TRN2 NeuronCores.

Problem: B=4, S=2048, D=1024, H=16 heads of DH=64, fp32 in/out,
causal + padding mask.

Sharding: core c -> (batch b = c//2, head-group g = c%2 of 8 heads).
Host converts activations/weights to fp16 (values are small; fp16 keeps
~1e-3 accuracy and runs the PE at 1 cycle/row vs ~2 for fp32r).

Per core:
    K^T = Wk_g @ X_kv^T   (512, 2048)  head dims on partitions   [kt tiles]
    Q^T = Wq_g @ X_q^T    (512, 2048)                            [qt tiles]
    V   = X_kv @ Wv_g^T   (2048, 512)  keys on partitions, with a
          leading ones column per head (softmax denominator trick) [vt]
    per head h, query half hh, key tile i (128 keys):
          S^T = K_h Q_h^T on a PSUM pair (keys on partitions)
          E = exp(S^T * scale + pad_bias)  -> fp16 SBUF
          diagonal 128x128 block causal-masked via affine_select
          Oaug^T += [ones|V_h]^T E        (PSUM rows: 0 = denom, 1..64 = O^T)
    normalize: recip(denom) on partition 0, partition_broadcast,
          multiply rows 1..64 -> staging, DMA into att row-blocks
    out^T_partial = Wo_g^T @ att^T  -> (1024, 2048), host sums the two
          group partials per batch and transposes.

Scheduling: the attention inner loop is software-pipelined per
(key-tile, query-half) step: scores for step s+1 are emitted before the
AV matmuls of step s, so the PE computes while the ACT engine runs exp.
The K/Q/V projection passes that are not needed up front are kept in a
need-by-ordered filler queue and popped between attention steps / at
half boundaries, filling PE gaps and keeping the tensor engine dense
(avoids DVFS downclocking seen on sparse PE streams).

PSUM (8 banks as 4 pairs A..D): AV accumulators on A (chunk parity),
score tiles ping-pong on C/D pairs, filler projection passes on B.
"""

import numpy as np

import concourse.bass as bass
import concourse.mybir as mybir
import concourse.tile as tile
from concourse import bacc

B, S, D, H = 4, 2048, 1024, 16
DH = 64
NG = 2              # head groups (cores per batch)
DG = D // NG        # 512 head dims per core
HL = H // NG        # 8 heads per core
PB = 128            # partition block
CH = 512            # free-dim chunk (one fp32 PSUM bank)
NCH = S // CH       # 4 chunks
NKT = S // PB       # 16 key tiles
NDT = D // PB       # 8 contraction tiles for projections
NJT = DG // PB      # 4 head-dim tiles per core
HS = S // 2         # 1024, half of seq
F32 = mybir.dt.float32
F16 = mybir.dt.float16
SCALE = 1.0 / 8.0   # 1/sqrt(DH)


def _emit(nc, xq, xkv, wq, wk, wv, wo, pb, outT):
    with tile.TileContext(nc) as tc:
        with (
            tc.tile_pool(name="pers", bufs=1) as pers,
            tc.tile_pool(name="xqp", bufs=1) as xqp,
            tc.tile_pool(name="xkp", bufs=1) as xkp,
            tc.tile_pool(name="wtp", bufs=1) as wtp,
            tc.tile_pool(name="qtp", bufs=1) as qtp,
            tc.tile_pool(name="ktp", bufs=1) as ktp,
            tc.tile_pool(name="vtp", bufs=1) as vtp,
            tc.tile_pool(name="atp", bufs=1) as atp,
            tc.tile_pool(name="exp", bufs=2) as exp_pool,
            tc.tile_pool(name="stg", bufs=2) as stgp,
            tc.tile_pool(name="rcp", bufs=2) as rcp,
            tc.tile_pool(name="ost", bufs=2) as ostp,
            tc.tile_pool(name="ps", bufs=1, space="PSUM") as ps,
        ):
            # ---------------- persistent small tiles ----------------
            pbias_sb = pers.tile([PB, NKT], F32, tag="pbias", name="pbias_sb")
            nc.sync.dma_start(out=pbias_sb[:], in_=pb[:].rearrange("(i p) -> p i", p=PB))

            # batched weight tiles: w*s[p, d*DG+f] = w*T[d*PB+p, f]
            wks = wtp.tile([PB, NDT * DG], F16, tag="wks", name="wks")
            wqs = wtp.tile([PB, NDT * DG], F16, tag="wqs", name="wqs")
            wvs = wtp.tile([PB, NDT * DG], F16, tag="wvs", name="wvs")
            wos = wtp.tile([PB, NJT * D], F16, tag="wos", name="wos")

            # single big x tiles: x*s[p, d*S + s] = x*T[d*PB+p, s]
            xkvs = xkp.tile([PB, NDT * S], F16, tag="xkv", name="xkvs")
            xqs = xqp.tile([PB, NDT * S], F16, tag="xqv", name="xqs")

            def load_w(dst, src, fsz, eng=None):
                (eng or nc.sync).dma_start(
                    out=dst[:].rearrange("p (d f) -> p d f", f=fsz),
                    in_=src[:].rearrange("(d p) f -> p d f", p=PB))

            def load_x(dst, src, c0, c1, eng=None):
                (eng or nc.sync).dma_start(
                    out=dst[:].rearrange("p (d s) -> p d s", s=S)[:, :, c0:c1],
                    in_=src[:, c0:c1].rearrange("(d p) f -> p d f", p=PB))

            # load order = first-use order, column-chunked so the first
            # projection passes start as early as possible
            load_w(wks, wk, DG)
            load_x(xkvs, xkv, 0, CH)
            load_x(xkvs, xkv, CH, HS)
            load_w(wqs, wq, DG)
            load_x(xqs, xq, 0, CH)
            load_x(xqs, xq, CH, HS)
            load_w(wvs, wv, DG)
            load_x(xkvs, xkv, HS, S)
            load_x(xqs, xq, HS, S)
            load_w(wos, wo, D)

            # ---------------- long-lived activation tiles ----------------
            qt = [qtp.tile([PB, S], F16, tag=f"qt{j}", name=f"qt{j}") for j in range(NJT)]
            kt = [ktp.tile([PB, S], F16, tag=f"kt{j}", name=f"kt{j}") for j in range(NJT)]
            # V with a LEADING ones column per head: [one | v(64)] x 8 heads
            vt = [vtp.tile([PB, HL * (DH + 1)], F16, tag=f"vt{i}", name=f"vt{i}") for i in range(NKT)]
            att = [atp.tile([PB, S], F16, tag=f"at{j}", name=f"at{j}") for j in range(NJT)]

            ones8 = pers.tile([PB, HL], F32, tag="ones8", name="ones8")
            nc.gpsimd.memset(ones8[:], 1.0)
            for i in range(NKT):
                ones_view = vt[i][:].rearrange("p (h c) -> p h c", c=DH + 1)[:, :, 0]
                nc.vector.tensor_copy(ones_view, ones8[:])

            # PSUM: four (128,1024) two-bank pairs
            pA = ps.tile([PB, 2 * CH], F32, tag="A", name="psA")
            pB = ps.tile([PB, 2 * CH], F32, tag="B", name="psB")
            pC = ps.tile([PB, 2 * CH], F32, tag="C", name="psC")
            pD = ps.tile([PB, 2 * CH], F32, tag="D", name="psD")
            A0, A1 = pA[:, 0:CH], pA[:, CH:2 * CH]
            B0, B1 = pB[:, 0:CH], pB[:, CH:2 * CH]
            D0, D1 = pD[:, 0:CH], pD[:, CH:2 * CH]
            C0, C1 = pC[:, 0:CH], pC[:, CH:2 * CH]

            # ---------------- projection pass emitters ----------------
            def kqproj_pass(ws, xs, dst, j, c, bank):
                # dst[j*PB:(j+1)*PB rows as partitions][:, c*CH:(c+1)*CH]
                for d in range(NDT):
                    nc.tensor.matmul(
                        bank,
                        ws[:, d * DG + j * PB:d * DG + (j + 1) * PB],
                        xs[:, d * S + c * CH:d * S + (c + 1) * CH],
                        start=(d == 0), stop=(d == NDT - 1),
                    )
                nc.vector.tensor_copy(dst[:, c * CH:(c + 1) * CH], bank)

            def vproj_pass(i, bank):
                for d in range(NDT):
                    nc.tensor.matmul(
                        bank,
                        xkvs[:, d * S + i * PB:d * S + (i + 1) * PB],
                        wvs[:, d * DG:(d + 1) * DG],
                        start=(d == 0), stop=(d == NDT - 1),
                    )
                src = bank.rearrange("p (h c) -> p h c", c=DH)
                dstv = vt[i][:].rearrange("p (h c) -> p h c", c=DH + 1)
                nc.vector.tensor_copy(dstv[:, :, 1:DH + 1], src)

            # ---------------- prefix: enough for head 0 half 0 ----------------
            pre_banks = [B0, B1, D0, D1]
            pre = []
            # all chunk-0 K passes first: they need only the first two
            # loads (wks + xkv cols 0:512), so they fill the PE while the
            # rest of the inputs stream in
            for j in range(NJT):
                pre.append(lambda b, j=j: kqproj_pass(wks, xkvs, kt[j], j, 0, b))
            pre.append(lambda b: kqproj_pass(wks, xkvs, kt[0], 0, 1, b))
            pre.append(lambda b: kqproj_pass(wks, xkvs, kt[1], 1, 1, b))
            pre.append(lambda b: kqproj_pass(wqs, xqs, qt[0], 0, 0, b))
            pre.append(lambda b: kqproj_pass(wqs, xqs, qt[0], 0, 1, b))
            for i in range(8):
                pre.append(lambda b, i=i: vproj_pass(i, b))
            for n, p in enumerate(pre):
                p(pre_banks[n % 4])

            # ---------------- filler queue, sorted by need-by ----------------
            # attention runs as two sweeps (all heads' first query halves,
            # then all second halves); need key = (hh, head, i) of the first
            # step that consumes the pass's output.
            fill = []

            def kq_need(j, ck):
                return (0, 2 * j, ck * 4) if ck < 2 else (1, 2 * j, (ck - 2) * 4 + 8)

            for j in range(NJT):
                for c in range(NCH):
                    if not (c == 0 or (c == 1 and j <= 1)):
                        fill.append((kq_need(j, c),
                                     lambda b, j=j, c=c: kqproj_pass(wks, xkvs, kt[j], j, c, b)))
                    if not (c < 2 and j == 0):
                        fill.append(((c // 2, 2 * j, 0),
                                     lambda b, j=j, c=c: kqproj_pass(wqs, xqs, qt[j], j, c, b)))
            for i in range(8, NKT):
                fill.append(((1, 0, i), lambda b, i=i: vproj_pass(i, b)))
            fill.sort(key=lambda e: e[0])

            fq = {"pos": 0, "bank": 0}
            fill_banks = [B0, B1]

            def pop_fill(n=1, need=None, max_key=None):
                while fq["pos"] < len(fill):
                    key, fn = fill[fq["pos"]]
                    if need is not None:
                        if key > need:
                            break
                    elif n <= 0 or (max_key is not None and key > max_key):
                        break
                    fn(fill_banks[fq["bank"] % 2])
                    fq["bank"] += 1
                    fq["pos"] += 1
                    n -= 1

            # ---------------- output projection plumbing ----------------
            # passes for query chunks 0/1 only need every head's first-half
            # att rows, which are staged by mid-head-7 -- so the c01 sweep is
            # interleaved into head 7's second half as PE filler; the c23
            # sweep runs at the tail.
            op_order = [(m, c) for c in range(NCH) for m in range(NDT)]
            op_state = {"pos": 0}

            def oproj_pass(bank):
                if op_state["pos"] >= len(op_order):
                    return
                m, c = op_order[op_state["pos"]]
                op_state["pos"] += 1
                for j in range(NJT):
                    nc.tensor.matmul(
                        bank,
                        wos[:, j * D + m * PB:j * D + (m + 1) * PB],
                        att[j][:, c * CH:(c + 1) * CH],
                        start=(j == 0), stop=(j == NJT - 1),
                    )
                oc = ostp.tile([PB, CH], F16, tag="oc", bufs=6, name="oc")
                nc.vector.tensor_copy(oc[:], bank)
                nc.sync.dma_start(
                    out=outT[m * PB:(m + 1) * PB, c * CH:(c + 1) * CH],
                    in_=oc[:])

            # ---------------- attention: two half-sweeps ----------------
            # sweep 0: all heads x query half 0; sweep 1: all heads x half 1.
            # This spreads the forced projection fillers across 64/128 steps
            # (instead of overloading head 0) and lets the output-projection
            # c01 passes weave through the whole second sweep.
            st_cnt = 0
            step_idx = 0
            for hh in range(2):
                for h in range(HL):
                    jq = h // 2
                    rowo = (h % 2) * DH
                    for i in range(8 if hh == 0 else NKT):
                        pop_fill(0, need=(hh, h, i))
                        q0 = max(i * PB, hh * HS)     # global query start
                        l0 = q0 - hh * HS             # local within half
                        st = [pC, pD][st_cnt % 2]
                        st_cnt += 1
                        for cl in range(l0 // CH, 2):
                            lo = max(l0, cl * CH)
                            nc.tensor.matmul(
                                st[:, lo:(cl + 1) * CH],
                                kt[jq][rowo:rowo + DH, i * PB:(i + 1) * PB],
                                qt[jq][rowo:rowo + DH, hh * HS + lo:hh * HS + (cl + 1) * CH],
                                start=True, stop=True,
                            )
                        ex_t = exp_pool.tile([PB, HS], F16, tag="ex", bufs=4, name="ex_t")
                        nc.scalar.activation(
                            ex_t[:, l0:HS], st[:, l0:HS],
                            mybir.ActivationFunctionType.Exp,
                            bias=pbias_sb[:, i:i + 1], scale=SCALE,
                        )
                        if i // 8 == hh:
                            # zero q < k inside the 128-wide diagonal block
                            db = i * PB - hh * HS
                            nc.gpsimd.affine_select(
                                out=ex_t[:, db:db + PB],
                                in_=ex_t[:, db:db + PB],
                                compare_op=mybir.AluOpType.is_ge, fill=0.0,
                                base=0, pattern=[[1, PB]],
                                channel_multiplier=-1,
                            )
                        # AV accumulation; diagonal chunk (lowest cl) last so
                        # the affine_select has drained by the time we need it
                        for cl in range(1, l0 // CH - 1, -1):
                            c = hh * 2 + cl
                            lo = max(l0, cl * CH)
                            bank = [A0, A1][c % 2]
                            nc.tensor.matmul(
                                bank[0:DH + 1, lo - cl * CH:CH],
                                vt[i][:, h * (DH + 1):(h + 1) * (DH + 1)],
                                ex_t[:, lo:(cl + 1) * CH],
                                start=(i == 0), stop=(i == 4 * c + 3),
                            )
                            if i == 4 * c + 3:
                                # copy raw [den|O^T] out of PSUM right away to
                                # release the AV bank for the next half/head,
                                # then normalize this query chunk: reciprocal
                                # runs at [128,4] (the DVE free dim is serial,
                                # a [1,512] recip costs ~3.3us) via DMA
                                # reshape, broadcast over partitions, multiply.
                                raw_t = rcp.tile([DH + 1, CH], F32, tag="raw", bufs=3, name="raw_t")
                                nc.vector.tensor_copy(raw_t[:], bank[0:DH + 1, :])
                                dnp_t = rcp.tile([PB, NCH], F32, tag="dnp", bufs=3, name="dnp_t")
                                nc.sync.dma_start(out=dnp_t[:], in_=raw_t[0:1, :])
                                rcs_t = rcp.tile([PB, NCH], F32, tag="rcs", bufs=3, name="rcs_t")
                                with nc.allow_low_precision(reason="softmax reciprocal"):
                                    nc.vector.reciprocal(rcs_t[:], dnp_t[:])
                                rc1_t = rcp.tile([1, CH], F32, tag="rc1", bufs=3, name="rc1_t")
                                nc.sync.dma_start(out=rc1_t[:], in_=rcs_t[:])
                                bc_t = rcp.tile([DH + 1, CH], F32, tag="bc", bufs=3, name="bc_t")
                                nc.gpsimd.partition_broadcast(bc_t[:], rc1_t[0:1, :])
                                # row 0 computes den*recip (unused); engine
                                # partition base must be 0/32/64/96
                                stg_t = stgp.tile([DH + 1, CH], F16, tag="stg", bufs=3, name="stg_t")
                                nc.vector.tensor_tensor(
                                    stg_t[:], raw_t[:], bc_t[:],
                                    mybir.AluOpType.mult,
                                )
                                nc.sync.dma_start(
                                    out=att[jq][rowo:rowo + DH, c * CH:(c + 1) * CH],
                                    in_=stg_t[1:DH + 1, :])
                        step_idx += 1
                        if hh == 0:
                            # sweep 0 consumes the kt/qt c01 passes; pull
                            # early sweep-1 passes in when they run dry
                            if step_idx % 4 == 2:
                                pop_fill(1, max_key=(1, 0, 99))
                        else:
                            s1 = step_idx - 64
                            if s1 >= 48 and s1 % 4 == 2 and op_state["pos"] < 16:
                                # oproj c01 only: every head's chunk-0/1 att
                                # rows are staged by the end of sweep 0. c23
                                # passes MUST stay behind all AV matmuls in PE
                                # program order (their att rows are produced
                                # by chains fed from later AV stops).
                                oproj_pass(fill_banks[fq["bank"] % 2])
                                fq["bank"] += 1
                            elif s1 % 4 == 0:
                                pop_fill(1, max_key=(1, h + 2, 99))
                    pop_fill(1)   # head boundary: cover the A-bank WAR gap

            pop_fill(len(fill))   # safety drain (normally empty here)

            # ---------------- output projection tail (c23 sweep) ----------------
            obanks = [B0, B1, C0, C1, D0, D1, A0, A1]
            ob = 0
            while op_state["pos"] < len(op_order):
                oproj_pass(obanks[ob % 8])
                ob += 1


def build_module():
    nc = bacc.Bacc()
    xq = nc.declare_dram_parameter("xqT", [D, S], F16, isOutput=False)
    xkv = nc.declare_dram_parameter("xkvT", [D, S], F16, isOutput=False)
    wq = nc.declare_dram_parameter("wqT", [D, DG], F16, isOutput=False)
    wk = nc.declare_dram_parameter("wkT", [D, DG], F16, isOutput=False)
    wv = nc.declare_dram_parameter("wvT", [D, DG], F16, isOutput=False)
    wo = nc.declare_dram_parameter("woT", [DG, D], F16, isOutput=False)
    pb = nc.declare_dram_parameter("pbias", [S], F32, isOutput=False)
    outT = nc.declare_dram_parameter("outT", [D, S], F16, isOutput=True)
    _emit(nc, xq, xkv, wq, wk, wv, wo, pb, outT)
    nc.finalize()
    return nc


_NC = None


def _get_nc():
    global _NC
    if _NC is None:
        _NC = build_module()
    return _NC


def make_in_maps(q_raw, kv_raw, padding_mask, Wq, Wk, Wv, Wo):
    q_raw = np.asarray(q_raw, np.float32)
    kv_raw = np.asarray(kv_raw, np.float32)
    qT = np.ascontiguousarray(q_raw.transpose(0, 2, 1)).astype(np.float16)
    kvT = np.ascontiguousarray(kv_raw.transpose(0, 2, 1)).astype(np.float16)
    pbias = np.where(np.asarray(padding_mask) == 0, -1e9, 0.0).astype(np.float32)
    Wq, Wk, Wv, Wo = (np.asarray(w, np.float32) for w in (Wq, Wk, Wv, Wo))
    wqT = [np.ascontiguousarray(Wq[g * DG:(g + 1) * DG, :].T).astype(np.float16) for g in range(NG)]
    wkT = [np.ascontiguousarray(Wk[g * DG:(g + 1) * DG, :].T).astype(np.float16) for g in range(NG)]
    wvT = [np.ascontiguousarray(Wv[g * DG:(g + 1) * DG, :].T).astype(np.float16) for g in range(NG)]
    woT = [np.ascontiguousarray(Wo[:, g * DG:(g + 1) * DG].T).astype(np.float16) for g in range(NG)]
    in_maps = []
    for c in range(NG * B):
        b, g = divmod(c, NG)
        in_maps.append({
            "xqT": qT[b], "xkvT": kvT[b],
            "wqT": wqT[g], "wkT": wkT[g], "wvT": wvT[g], "woT": woT[g],
            "pbias": pbias[b],
        })
    return in_maps


def kernel(q_raw, kv_raw, padding_mask, Wq, Wk, Wv, Wo):
    from concourse.bass_utils import run_bass_kernel_spmd

    nc = _get_nc()
    in_maps = make_in_maps(q_raw, kv_raw, padding_mask, Wq, Wk, Wv, Wo)
    res = run_bass_kernel_spmd(nc, in_maps, core_ids=list(range(NG * B)))
    out = np.empty((B, S, D), np.float32)
    for b in range(B):
        out[b] = (res.results[NG * b]["outT"].astype(np.float32)
                  + res.results[NG * b + 1]["outT"].astype(np.float32)).T
    return out


# revision 52
# speedup vs baseline: 1.1858x; 1.1858x over previous
"""Distributed causal multi-head attention for 8 TRN2 NeuronCores.

Problem: B=4, S=2048, D=1024, H=16 heads of DH=64, fp32 in/out,
causal + padding mask.

Sharding: core c -> (batch b = c//2, head-group g = c%2 of 8 heads).
Host converts activations/weights to fp16 (values are small; fp16 keeps
~1e-3 accuracy and runs the PE at 1 cycle/row vs ~2 for fp32r).

Per core:
    K^T = Wk_g @ X_kv^T   (512, 2048)  head dims on partitions   [kt tiles]
    Q^T = Wq_g @ X_q^T    (512, 2048)                            [qt tiles]
    V   = X_kv @ Wv_g^T   (2048, 512)  keys on partitions, with a
          leading ones column per head (softmax denominator trick) [vt]
    per head h, query half hh, key tile i (128 keys):
          S^T = K_h Q_h^T on a PSUM pair (keys on partitions)
          E = exp(S^T * scale + pad_bias)  -> fp16 SBUF
          diagonal 128x128 block causal-masked via affine_select
          Oaug^T += [ones|V_h]^T E        (PSUM rows: 0 = denom, 1..64 = O^T)
    normalize: recip(denom) on partition 0, partition_broadcast,
          multiply rows 1..64 -> staging, DMA into att row-blocks
    out^T_partial = Wo_g^T @ att^T  -> (1024, 2048), host sums the two
          group partials per batch and transposes.

Scheduling: the attention inner loop is software-pipelined per
(key-tile, query-half) step: scores for step s+1 are emitted before the
AV matmuls of step s, so the PE computes while the ACT engine runs exp.
The K/Q/V projection passes that are not needed up front are kept in a
need-by-ordered filler queue and popped between attention steps / at
half boundaries, filling PE gaps and keeping the tensor engine dense
(avoids DVFS downclocking seen on sparse PE streams).

PSUM (8 banks as 4 pairs A..D): AV accumulators on A (chunk parity),
score tiles ping-pong on C/D pairs, filler projection passes on B.
"""

import numpy as np

import concourse.bass as bass
import concourse.mybir as mybir
import concourse.tile as tile
from concourse import bacc

B, S, D, H = 4, 2048, 1024, 16
DH = 64
NG = 2              # head groups (cores per batch)
DG = D // NG        # 512 head dims per core
HL = H // NG        # 8 heads per core
PB = 128            # partition block
CH = 512            # free-dim chunk (one fp32 PSUM bank)
NCH = S // CH       # 4 chunks
NKT = S // PB       # 16 key tiles
NDT = D // PB       # 8 contraction tiles for projections
NJT = DG // PB      # 4 head-dim tiles per core
HS = S // 2         # 1024, half of seq
F32 = mybir.dt.float32
F16 = mybir.dt.float16
SCALE = 1.0 / 8.0   # 1/sqrt(DH)


def _emit(nc, xq, xkv, wq, wk, wv, wo, pb, outT):
    with tile.TileContext(nc) as tc:
        with (
            tc.tile_pool(name="pers", bufs=1) as pers,
            tc.tile_pool(name="xqp", bufs=1) as xqp,
            tc.tile_pool(name="xkp", bufs=1) as xkp,
            tc.tile_pool(name="wtp", bufs=1) as wtp,
            tc.tile_pool(name="qtp", bufs=1) as qtp,
            tc.tile_pool(name="ktp", bufs=1) as ktp,
            tc.tile_pool(name="vtp", bufs=1) as vtp,
            tc.tile_pool(name="atp", bufs=1) as atp,
            tc.tile_pool(name="exp", bufs=2) as exp_pool,
            tc.tile_pool(name="stg", bufs=2) as stgp,
            tc.tile_pool(name="rcp", bufs=2) as rcp,
            tc.tile_pool(name="ost", bufs=2) as ostp,
            tc.tile_pool(name="ps", bufs=1, space="PSUM") as ps,
        ):
            # ---------------- persistent small tiles ----------------
            pbias_sb = pers.tile([PB, NKT], F32, tag="pbias", name="pbias_sb")
            nc.sync.dma_start(out=pbias_sb[:], in_=pb[:].rearrange("(i p) -> p i", p=PB))

            # batched weight tiles: w*s[p, d*DG+f] = w*T[d*PB+p, f]
            wks = wtp.tile([PB, NDT * DG], F16, tag="wks", name="wks")
            wqs = wtp.tile([PB, NDT * DG], F16, tag="wqs", name="wqs")
            wvs = wtp.tile([PB, NDT * DG], F16, tag="wvs", name="wvs")
            wos = wtp.tile([PB, NJT * D], F16, tag="wos", name="wos")

            # single big x tiles: x*s[p, d*S + s] = x*T[d*PB+p, s]
            xkvs = xkp.tile([PB, NDT * S], F16, tag="xkv", name="xkvs")
            xqs = xqp.tile([PB, NDT * S], F16, tag="xqv", name="xqs")

            def load_w(dst, src, fsz, eng=None):
                (eng or nc.sync).dma_start(
                    out=dst[:].rearrange("p (d f) -> p d f", f=fsz),
                    in_=src[:].rearrange("(d p) f -> p d f", p=PB))

            def load_x(dst, src, c0, c1, eng=None):
                (eng or nc.sync).dma_start(
                    out=dst[:].rearrange("p (d s) -> p d s", s=S)[:, :, c0:c1],
                    in_=src[:, c0:c1].rearrange("(d p) f -> p d f", p=PB))

            # load order = first-use order, column-chunked so the first
            # projection passes start as early as possible
            load_w(wks, wk, DG)
            load_x(xkvs, xkv, 0, CH)
            load_x(xkvs, xkv, CH, HS)
            load_w(wqs, wq, DG)
            load_x(xqs, xq, 0, CH)
            load_x(xqs, xq, CH, HS)
            load_w(wvs, wv, DG)
            load_x(xkvs, xkv, HS, S)
            load_x(xqs, xq, HS, S)
            load_w(wos, wo, D)

            # ---------------- long-lived activation tiles ----------------
            qt = [qtp.tile([PB, S], F16, tag=f"qt{j}", name=f"qt{j}") for j in range(NJT)]
            kt = [ktp.tile([PB, S], F16, tag=f"kt{j}", name=f"kt{j}") for j in range(NJT)]
            # V with a LEADING ones column per head: [one | v(64)] x 8 heads
            vt = [vtp.tile([PB, HL * (DH + 1)], F16, tag=f"vt{i}", name=f"vt{i}") for i in range(NKT)]
            att = [atp.tile([PB, S], F16, tag=f"at{j}", name=f"at{j}") for j in range(NJT)]

            ones8 = pers.tile([PB, HL], F32, tag="ones8", name="ones8")
            nc.gpsimd.memset(ones8[:], 1.0)
            for i in range(NKT):
                ones_view = vt[i][:].rearrange("p (h c) -> p h c", c=DH + 1)[:, :, 0]
                nc.vector.tensor_copy(ones_view, ones8[:])

            # PSUM: four (128,1024) two-bank pairs
            pA = ps.tile([PB, 2 * CH], F32, tag="A", name="psA")
            pB = ps.tile([PB, 2 * CH], F32, tag="B", name="psB")
            pC = ps.tile([PB, 2 * CH], F32, tag="C", name="psC")
            pD = ps.tile([PB, 2 * CH], F32, tag="D", name="psD")
            A0, A1 = pA[:, 0:CH], pA[:, CH:2 * CH]
            B0, B1 = pB[:, 0:CH], pB[:, CH:2 * CH]
            D0, D1 = pD[:, 0:CH], pD[:, CH:2 * CH]
            C0, C1 = pC[:, 0:CH], pC[:, CH:2 * CH]

            # ---------------- projection pass emitters ----------------
            def kqproj_pass(ws, xs, dst, j, c, bank):
                # dst[j*PB:(j+1)*PB rows as partitions][:, c*CH:(c+1)*CH]
                for d in range(NDT):
                    nc.tensor.matmul(
                        bank,
                        ws[:, d * DG + j * PB:d * DG + (j + 1) * PB],
                        xs[:, d * S + c * CH:d * S + (c + 1) * CH],
                        start=(d == 0), stop=(d == NDT - 1),
                    )
                nc.vector.tensor_copy(dst[:, c * CH:(c + 1) * CH], bank)

            def vproj_pass(i, bank):
                for d in range(NDT):
                    nc.tensor.matmul(
                        bank,
                        xkvs[:, d * S + i * PB:d * S + (i + 1) * PB],
                        wvs[:, d * DG:(d + 1) * DG],
                        start=(d == 0), stop=(d == NDT - 1),
                    )
                src = bank.rearrange("p (h c) -> p h c", c=DH)
                dstv = vt[i][:].rearrange("p (h c) -> p h c", c=DH + 1)
                nc.vector.tensor_copy(dstv[:, :, 1:DH + 1], src)

            # ---------------- prefix: enough for head 0 half 0 ----------------
            pre_banks = [B0, B1, D0, D1]
            pre = []
            # all chunk-0 K passes first: they need only the first two
            # loads (wks + xkv cols 0:512), so they fill the PE while the
            # rest of the inputs stream in
            for j in range(NJT):
                pre.append(lambda b, j=j: kqproj_pass(wks, xkvs, kt[j], j, 0, b))
            pre.append(lambda b: kqproj_pass(wks, xkvs, kt[0], 0, 1, b))
            pre.append(lambda b: kqproj_pass(wks, xkvs, kt[1], 1, 1, b))
            pre.append(lambda b: kqproj_pass(wqs, xqs, qt[0], 0, 0, b))
            pre.append(lambda b: kqproj_pass(wqs, xqs, qt[0], 0, 1, b))
            for i in range(8):
                pre.append(lambda b, i=i: vproj_pass(i, b))
            for n, p in enumerate(pre):
                p(pre_banks[n % 4])

            # ---------------- filler queue, sorted by need-by ----------------
            # attention runs as two sweeps (all heads' first query halves,
            # then all second halves); need key = (hh, head, i) of the first
            # step that consumes the pass's output.
            fill = []

            def kq_need(j, ck):
                return (0, 2 * j, ck * 4) if ck < 2 else (1, 2 * j, (ck - 2) * 4 + 8)

            for j in range(NJT):
                for c in range(NCH):
                    if not (c == 0 or (c == 1 and j <= 1)):
                        fill.append((kq_need(j, c),
                                     lambda b, j=j, c=c: kqproj_pass(wks, xkvs, kt[j], j, c, b)))
                    if not (c < 2 and j == 0):
                        fill.append(((c // 2, 2 * j, 0),
                                     lambda b, j=j, c=c: kqproj_pass(wqs, xqs, qt[j], j, c, b)))
            for i in range(8, NKT):
                fill.append(((1, 0, i), lambda b, i=i: vproj_pass(i, b)))
            fill.sort(key=lambda e: e[0])

            fq = {"pos": 0, "bank": 0}
            fill_banks = [B0, B1]

            def pop_fill(n=1, need=None, max_key=None):
                while fq["pos"] < len(fill):
                    key, fn = fill[fq["pos"]]
                    if need is not None:
                        if key > need:
                            break
                    elif n <= 0 or (max_key is not None and key > max_key):
                        break
                    fn(fill_banks[fq["bank"] % 2])
                    fq["bank"] += 1
                    fq["pos"] += 1
                    n -= 1

            # ---------------- output projection plumbing ----------------
            # passes for query chunks 0/1 only need every head's first-half
            # att rows, which are staged by mid-head-7 -- so the c01 sweep is
            # interleaved into head 7's second half as PE filler; the c23
            # sweep runs at the tail.
            op_order = [(m, c) for c in range(NCH) for m in range(NDT)]
            op_state = {"pos": 0}

            def oproj_pass(bank):
                if op_state["pos"] >= len(op_order):
                    return
                m, c = op_order[op_state["pos"]]
                op_state["pos"] += 1
                for j in range(NJT):
                    nc.tensor.matmul(
                        bank,
                        wos[:, j * D + m * PB:j * D + (m + 1) * PB],
                        att[j][:, c * CH:(c + 1) * CH],
                        start=(j == 0), stop=(j == NJT - 1),
                    )
                oc = ostp.tile([PB, CH], F16, tag="oc", bufs=6, name="oc")
                nc.vector.tensor_copy(oc[:], bank)
                nc.sync.dma_start(
                    out=outT[m * PB:(m + 1) * PB, c * CH:(c + 1) * CH],
                    in_=oc[:])

            # ---------------- attention: two half-sweeps ----------------
            # sweep 0: all heads x query half 0; sweep 1: all heads x half 1.
            # This spreads the forced projection fillers across 64/128 steps
            # (instead of overloading head 0) and lets the output-projection
            # c01 passes weave through the whole second sweep.
            st_cnt = 0
            step_idx = 0
            for hh in range(2):
                for h in range(HL):
                    jq = h // 2
                    rowo = (h % 2) * DH
                    for i in range(8 if hh == 0 else NKT):
                        pop_fill(0, need=(hh, h, i))
                        q0 = max(i * PB, hh * HS)     # global query start
                        l0 = q0 - hh * HS             # local within half
                        st = [pC, pD][st_cnt % 2]
                        st_cnt += 1
                        for cl in range(l0 // CH, 2):
                            lo = max(l0, cl * CH)
                            nc.tensor.matmul(
                                st[:, lo:(cl + 1) * CH],
                                kt[jq][rowo:rowo + DH, i * PB:(i + 1) * PB],
                                qt[jq][rowo:rowo + DH, hh * HS + lo:hh * HS + (cl + 1) * CH],
                                start=True, stop=True,
                            )
                        ex_t = exp_pool.tile([PB, HS], F16, tag="ex", bufs=4, name="ex_t")
                        nc.scalar.activation(
                            ex_t[:, l0:HS], st[:, l0:HS],
                            mybir.ActivationFunctionType.Exp,
                            bias=pbias_sb[:, i:i + 1], scale=SCALE,
                        )
                        if i // 8 == hh:
                            # zero q < k inside the 128-wide diagonal block
                            db = i * PB - hh * HS
                            nc.gpsimd.affine_select(
                                out=ex_t[:, db:db + PB],
                                in_=ex_t[:, db:db + PB],
                                compare_op=mybir.AluOpType.is_ge, fill=0.0,
                                base=0, pattern=[[1, PB]],
                                channel_multiplier=-1,
                            )
                        # AV accumulation; diagonal chunk (lowest cl) last so
                        # the affine_select has drained by the time we need it
                        for cl in range(1, l0 // CH - 1, -1):
                            c = hh * 2 + cl
                            lo = max(l0, cl * CH)
                            bank = [A0, A1][c % 2]
                            nc.tensor.matmul(
                                bank[0:DH + 1, lo - cl * CH:CH],
                                vt[i][:, h * (DH + 1):(h + 1) * (DH + 1)],
                                ex_t[:, lo:(cl + 1) * CH],
                                start=(i == 0), stop=(i == 4 * c + 3),
                            )
                            if i == 4 * c + 3:
                                # copy raw [den|O^T] out of PSUM right away to
                                # release the AV bank for the next half/head,
                                # then normalize this query chunk: reciprocal
                                # runs at [128,4] (the DVE free dim is serial,
                                # a [1,512] recip costs ~3.3us) via DMA
                                # reshape, broadcast over partitions, multiply.
                                raw_t = rcp.tile([DH + 1, CH], F32, tag="raw", bufs=3, name="raw_t")
                                nc.vector.tensor_copy(raw_t[:], bank[0:DH + 1, :])
                                dnp_t = rcp.tile([PB, NCH], F32, tag="dnp", bufs=3, name="dnp_t")
                                nc.sync.dma_start(out=dnp_t[:], in_=raw_t[0:1, :])
                                rcs_t = rcp.tile([PB, NCH], F32, tag="rcs", bufs=3, name="rcs_t")
                                with nc.allow_low_precision(reason="softmax reciprocal"):
                                    nc.vector.reciprocal(rcs_t[:], dnp_t[:])
                                rc1_t = rcp.tile([1, CH], F32, tag="rc1", bufs=3, name="rc1_t")
                                nc.sync.dma_start(out=rc1_t[:], in_=rcs_t[:])
                                bc_t = rcp.tile([DH + 1, CH], F32, tag="bc", bufs=3, name="bc_t")
                                nc.gpsimd.partition_broadcast(bc_t[:], rc1_t[0:1, :])
                                # row 0 computes den*recip (unused); engine
                                # partition base must be 0/32/64/96
                                stg_t = stgp.tile([DH + 1, CH], F16, tag="stg", bufs=3, name="stg_t")
                                nc.vector.tensor_tensor(
                                    stg_t[:], raw_t[:], bc_t[:],
                                    mybir.AluOpType.mult,
                                )
                                nc.sync.dma_start(
                                    out=att[jq][rowo:rowo + DH, c * CH:(c + 1) * CH],
                                    in_=stg_t[1:DH + 1, :])
                        step_idx += 1
                        if hh == 0:
                            # sweep 0 consumes the kt/qt c01 passes; pull
                            # early sweep-1 passes in when they run dry
                            if step_idx % 4 == 2:
                                pop_fill(1, max_key=(1, 0, 99))
                        else:
                            s1 = step_idx - 64
                            if s1 >= 48 and s1 % 4 == 2 and op_state["pos"] < 16:
                                # oproj c01 only: every head's chunk-0/1 att
                                # rows are staged by the end of sweep 0. c23
                                # passes MUST stay behind all AV matmuls in PE
                                # program order (their att rows are produced
                                # by chains fed from later AV stops).
                                oproj_pass(fill_banks[fq["bank"] % 2])
                                fq["bank"] += 1
                            elif s1 % 4 == 0:
                                pop_fill(1, max_key=(1, h + 2, 99))
                    pop_fill(1)   # head boundary: cover the A-bank WAR gap

            pop_fill(len(fill))   # safety drain (normally empty here)

            # ---------------- output projection tail (c23 sweep) ----------------
            obanks = [B0, B1, C0, C1, D0, D1, A0, A1]
            ob = 0
            while op_state["pos"] < len(op_order):
                oproj_pass(obanks[ob % 8])
                ob += 1


def build_module():
    nc = bacc.Bacc()
    xq = nc.declare_dram_parameter("xqT", [D, S], F16, isOutput=False)
    xkv = nc.declare_dram_parameter("xkvT", [D, S], F16, isOutput=False)
    wq = nc.declare_dram_parameter("wqT", [D, DG], F16, isOutput=False)
    wk = nc.declare_dram_parameter("wkT", [D, DG], F16, isOutput=False)
    wv = nc.declare_dram_parameter("wvT", [D, DG], F16, isOutput=False)
    wo = nc.declare_dram_parameter("woT", [DG, D], F16, isOutput=False)
    pb = nc.declare_dram_parameter("pbias", [S], F32, isOutput=False)
    outT = nc.declare_dram_parameter("outT", [D, S], F16, isOutput=True)
    _emit(nc, xq, xkv, wq, wk, wv, wo, pb, outT)
    nc.finalize()
    return nc


_NC = None


def _get_nc():
    global _NC
    if _NC is None:
        _NC = build_module()
    return _NC


def make_in_maps(q_raw, kv_raw, padding_mask, Wq, Wk, Wv, Wo):
    q_raw = np.asarray(q_raw, np.float32)
    kv_raw = np.asarray(kv_raw, np.float32)
    qT = np.ascontiguousarray(q_raw.transpose(0, 2, 1)).astype(np.float16)
    kvT = np.ascontiguousarray(kv_raw.transpose(0, 2, 1)).astype(np.float16)
    pbias = np.where(np.asarray(padding_mask) == 0, -1e9, 0.0).astype(np.float32)
    Wq, Wk, Wv, Wo = (np.asarray(w, np.float32) for w in (Wq, Wk, Wv, Wo))
    wqT = [np.ascontiguousarray(Wq[g * DG:(g + 1) * DG, :].T).astype(np.float16) for g in range(NG)]
    wkT = [np.ascontiguousarray(Wk[g * DG:(g + 1) * DG, :].T).astype(np.float16) for g in range(NG)]
    wvT = [np.ascontiguousarray(Wv[g * DG:(g + 1) * DG, :].T).astype(np.float16) for g in range(NG)]
    woT = [np.ascontiguousarray(Wo[:, g * DG:(g + 1) * DG].T).astype(np.float16) for g in range(NG)]
    in_maps = []
    for c in range(NG * B):
        b, g = divmod(c, NG)
        in_maps.append({
            "xqT": qT[b], "xkvT": kvT[b],
            "wqT": wqT[g], "wkT": wkT[g], "wvT": wvT[g], "woT": woT[g],
            "pbias": pbias[b],
        })
    return in_maps


def kernel(q_raw, kv_raw, padding_mask, Wq, Wk, Wv, Wo):
    from concourse.bass_utils import run_bass_kernel_spmd

    nc = _get_nc()
    in_maps = make_in_maps(q_raw, kv_raw, padding_mask, Wq, Wk, Wv, Wo)
    res = run_bass_kernel_spmd(nc, in_maps, core_ids=list(range(NG * B)))
    out = np.empty((B, S, D), np.float32)
    for b in range(B):
        out[b] = (res.results[NG * b]["outT"].astype(np.float32)
                  + res.results[NG * b + 1]["outT"].astype(np.float32)).T
    return out
